# revision 1
# baseline (speedup 1.0000x reference)
"""Trainium2 Bass kernel for nn_Encoder_P: unwrap-diff-square front-end + 4 dilated
convs with dense concatenation, fused end-to-end on-chip.

Strategy (pure data parallel, 1 batch sample per NeuronCore, 8 cores):
  - The unwrap/diff/pad chain collapses: cumsum cancels in the diff, so
    sq[h] = wrap(p[h] - p[h-1])^2 (row 0 = 0), wrap(v) = v - 2*pi*k with
    k = (v>=pi) + (v>=3pi) - (v<=-pi) - (v<=-3pi).
  - Duplicate concat channels are folded into effective conv weights
    (conv3: 8->7 input planes, conv4: 20->15).
  - Each conv runs on TensorE as banded matmuls over the H (partition) axis:
    lhsT is a banded [128,128] H-shift matrix built on-device (DVE) from 5
    shared shifted-identity masters scaled by runtime weight scalars; rhs is
    the input plane tile [128 H, 516 Wpad]; PSUM accumulates over (ci, kw).
  - Planes are stored as 5 overlapping H-tiles (stride 104, halo 12) of
    [128, 516] with zeroed W margins, so conv H/W reach never crosses a tile.
"""

import numpy as np

import concourse.bacc as bacc
import concourse.bass as bass
import concourse.mybir as mybir
import concourse.tile as tile
from concourse import bass_utils

F32 = mybir.dt.float32
MM_DT = mybir.dt.float32r  # full-rate fp32 matmul path (1 cyc/row at N>=256)
DEFAULT_MM = "f32r"  # flip to "bf16" only with HW-validated accuracy+speed

H = 512
W = 512
S = 107          # tile stride in rows (chosen so 512-(S*4-HALO) == 96, a legal
                 # compute-op partition start for the bottom edge-zero memset)
HALO = 12        # halo rows above/below each tile
NT = 5           # number of H tiles
WPAD = 516       # 2 zero cols + 512 + 2 zero cols
P = 128
PI = float(np.pi)

# conv specs: (dil, pad_top, pad_left, KH, KW)
CONV_GEOM = [
    (1, 1, 1, 4, 4),   # conv1: 4x4 dil1, 'same' pad (1,2)
    (2, 2, 2, 3, 3),   # conv2: 3x3 dil2, pad (2,2)
    (3, 1, 1, 2, 2),   # conv3: 2x2 dil3, pad (1,2)
    (4, 0, 0, 1, 1),   # conv4: 1x1
]

PLANE_NAMES = (
    ["sq", "c1_0", "c1_1"]
    + [f"c2_{i}" for i in range(4)]
    + [f"c3_{i}" for i in range(8)]
)
CONV_INPUTS = [
    ["sq"],
    ["c1_0", "c1_1", "sq"],
    [f"c2_{i}" for i in range(4)] + ["c1_0", "c1_1", "sq"],
    [f"c3_{i}" for i in range(8)] + [f"c2_{i}" for i in range(4)]
    + ["c1_0", "c1_1", "sq"],
]
CONV_OUT = [2, 4, 8, 16]
DELTAS = [-2, -1, 0, 1, 2]  # identity master shifts

# output channel -> source plane ("c4_o" channels handled separately)
CH_MAP = (
    [f"c4_{i}" for i in range(16)]
    + [f"c3_{i}" for i in range(8)]
    + [f"c2_{i}" for i in range(4)]
    + ["c1_0", "c1_1", "sq", "sq", "c1_0", "c1_1", "sq", "sq"]
    + [f"c2_{i}" for i in range(4)]
    + ["c1_0", "c1_1", "sq", "sq"]
    + ["c1_0", "c1_1", "sq", "sq"]
)

NSCAL = sum(
    CONV_OUT[c] * len(CONV_INPUTS[c]) * CONV_GEOM[c][3] * CONV_GEOM[c][4]
    for c in range(4)
)  # 604


def _fold_weights(w1, w2, w3, w4):
    w3f = np.zeros((8, 7, 2, 2), np.float32)
    w3f[:, :6] = w3[:, :6]
    w3f[:, 6] = w3[:, 6] + w3[:, 7]
    w4f = np.zeros((16, 15, 1, 1), np.float32)
    w4f[:, :12] = w4[:, :12]
    w4f[:, 12] = w4[:, 12] + w4[:, 16]
    w4f[:, 13] = w4[:, 13] + w4[:, 17]
    w4f[:, 14] = w4[:, 14] + w4[:, 15] + w4[:, 18] + w4[:, 19]
    return [w1.astype(np.float32), w2.astype(np.float32), w3f, w4f]


def _host_tables(inputs):
    """wtab [128, NSCAL], ident [5*128, 128], bias [128, 30] host arrays."""
    wf = _fold_weights(inputs["w1"], inputs["w2"], inputs["w3"], inputs["w4"])
    scal = []
    for c in range(4):
        dil, pad_top, _, KH, KW = CONV_GEOM[c]
        for o in range(CONV_OUT[c]):
            for ci in range(len(CONV_INPUTS[c])):
                for kw in range(KW):
                    for kh in range(KH):
                        scal.append(wf[c][o, ci, kh, kw])
    assert len(scal) == NSCAL
    wtab = np.tile(np.asarray(scal, np.float32)[None, :], (P, 1))
    ident = np.concatenate(
        [np.eye(P, dtype=np.float32, k=-d) for d in DELTAS], axis=0
    )
    bias = np.concatenate(
        [inputs["b1"], inputs["b2"], inputs["b3"], inputs["b4"]]
    ).astype(np.float32)
    bias = np.tile(bias[None, :], (P, 1))
    return wtab, ident, bias


def build_nc(loop_k=1, out_mode='full', skip_bands=False, mm='f32r'):
    nc = bacc.Bacc("TRN2", target_bir_lowering=False, debug=False)
    mm_dt = mybir.dt.bfloat16 if mm == 'bf16' else MM_DT

    def msafe(ap):
        # memset target: walrus rejects float32r memsets; bitcast those to f32
        return ap.bitcast(F32) if mm != 'bf16' else ap

    p_dram = nc.dram_tensor("p", [H, W], F32, kind="ExternalInput")
    ident_dram = nc.dram_tensor("ident", [5 * P, P], F32, kind="ExternalInput")
    wtab_dram = nc.dram_tensor("wtab", [P, NSCAL], F32, kind="ExternalInput")
    bias_dram = nc.dram_tensor("bias", [P, 30], F32, kind="ExternalInput")
    out_dram = nc.dram_tensor("out", [48, H, W], F32, kind="ExternalOutput")

    planes = {
        nm: nc.alloc_sbuf_tensor(f"pl_{nm}", [P, NT * WPAD], mm_dt)
        for nm in PLANE_NAMES
    }
    ident_sb = nc.alloc_sbuf_tensor("ident_sb", [P, 5 * P], F32)
    wtab_sb = nc.alloc_sbuf_tensor("wtab_sb", [P, NSCAL], F32)
    bias_sb = nc.alloc_sbuf_tensor("bias_sb", [P, 30], F32)

    def pslice(nm, t, c0, c1):
        return planes[nm][:, t * WPAD + c0 : t * WPAD + c1]

    with tile.TileContext(nc) as tc:
        with (
            tc.tile_pool(name="io", bufs=3) as io_pool,
            tc.tile_pool(name="front", bufs=2) as fr_pool,
            tc.tile_pool(name="bands", bufs=12) as band_pool,
            tc.tile_pool(name="psum", bufs=8, space="PSUM") as psum_pool,
            tc.tile_pool(name="c4st", bufs=3) as c4_pool,
        ):
            for _it in range(loop_k):
                # ---- parameter loads ----
                for j in range(5):
                    nc.sync.dma_start(
                        out=ident_sb[:, j * P : (j + 1) * P],
                        in_=ident_dram[j * P : (j + 1) * P, :],
                    )
                nc.sync.dma_start(out=wtab_sb[:], in_=wtab_dram[:])
                nc.sync.dma_start(out=bias_sb[:], in_=bias_dram[:])

                # ---- zero W margins of all planes (written once) ----
                for nm in PLANE_NAMES:
                    for t in range(NT):
                        nc.gpsimd.memset(msafe(pslice(nm, t, 0, 2)), 0.0)
                        nc.gpsimd.memset(msafe(pslice(nm, t, 514, 516)), 0.0)

                # ---- front-end: sq ----
                # A/B garbage regions are pre-zeroed so the out-of-image rows
                # compute v=0 -> sq=0, which is exactly the reference's zero pad.
                for t in range(NT):
                    p_lo = HALO if t == 0 else 0
                    p_hi = H - (S * (NT - 1) - HALO) if t == NT - 1 else P  # 96 at t=4
                    n = p_hi - p_lo
                    r_lo = S * t - HALO + p_lo
                    A = io_pool.tile([P, W], F32, tag="A")
                    B = io_pool.tile([P, W], F32, tag="B")
                    if t == 0:
                        nc.gpsimd.memset(A[0:32, :], 0.0)
                        nc.gpsimd.memset(B[0:32, :], 0.0)
                    if t == NT - 1:
                        nc.gpsimd.memset(A[96:P, :], 0.0)
                        nc.gpsimd.memset(B[96:P, :], 0.0)
                    nc.sync.dma_start(out=A[p_lo:p_hi, :], in_=p_dram[r_lo : r_lo + n, :])
                    if t == 0:
                        nc.sync.dma_start(
                            out=B[p_lo + 1 : p_hi, :], in_=p_dram[0 : n - 1, :]
                        )
                        nc.sync.dma_start(out=B[p_lo : p_lo + 1, :], in_=p_dram[0:1, :])
                    else:
                        nc.sync.dma_start(
                            out=B[p_lo:p_hi, :], in_=p_dram[r_lo - 1 : r_lo - 1 + n, :]
                        )
                    V = fr_pool.tile([P, W], F32, tag="V")
                    K1 = fr_pool.tile([P, W], F32, tag="K1")
                    K2 = fr_pool.tile([P, W], F32, tag="K2")
                    K3 = fr_pool.tile([P, W], F32, tag="K3")
                    K4 = fr_pool.tile([P, W], F32, tag="K4")
                    ao = mybir.AluOpType
                    nc.vector.tensor_tensor(V[:], A[:], B[:], ao.subtract)
                    nc.vector.tensor_scalar(K1[:], V[:], PI, None, ao.is_ge)
                    nc.vector.tensor_scalar(K2[:], V[:], 3 * PI, None, ao.is_ge)
                    nc.vector.tensor_scalar(K3[:], V[:], -PI, None, ao.is_le)
                    nc.vector.tensor_scalar(K4[:], V[:], -3 * PI, None, ao.is_le)
                    nc.vector.tensor_tensor(K1[:], K1[:], K2[:], ao.add)
                    nc.vector.tensor_tensor(K3[:], K3[:], K4[:], ao.add)
                    nc.vector.tensor_tensor(K1[:], K1[:], K3[:], ao.subtract)
                    nc.vector.scalar_tensor_tensor(
                        V[:], K1[:], -2 * PI, V[:], ao.mult, ao.add
                    )
                    sq_dst = planes["sq"][:, t * WPAD + 2 : t * WPAD + 514]
                    nc.vector.tensor_tensor(sq_dst, V[:], V[:], ao.mult)

                # ---- convs ----
                jcol = 0
                bias_col = 0
                p_hi_last = H - (S * (NT - 1) - HALO)  # 108
                for c in range(4):
                    dil, pad_top, pad_left, KH, KW = CONV_GEOM[c]
                    in_names = CONV_INPUTS[c]
                    O = CONV_OUT[c]
                    deltas = [kh * dil - pad_top for kh in range(KH)]
                    for o in range(O):
                        psums = [
                            psum_pool.tile([P, W], F32, tag="ps", name=f"ps_{c}_{o}_{t}")
                            for t in range(NT)
                        ]
                        for ci, nm in enumerate(in_names):
                            for kw in range(KW):
                                band = band_pool.tile([P, P], mm_dt, tag="band")
                                if skip_bands:
                                    deltas_eff = []
                                    jcol += len(deltas)
                                else:
                                    deltas_eff = deltas
                                for i, d in enumerate(deltas_eff):
                                    w_ap = wtab_sb[:, jcol : jcol + 1]
                                    jcol += 1
                                    src = ident_sb[
                                        :, (d + 2) * P : (d + 3) * P
                                    ]
                                    ao = mybir.AluOpType
                                    if i == 0:
                                        nc.vector.tensor_scalar(
                                            band[:], src, w_ap, None, ao.mult
                                        )
                                    else:
                                        nc.vector.scalar_tensor_tensor(
                                            band[:], src, w_ap, band[:], ao.mult, ao.add
                                        )
                                coff = 2 + kw * dil - pad_left
                                first = ci == 0 and kw == 0
                                last = ci == len(in_names) - 1 and kw == KW - 1
                                for t in range(NT):
                                    rhs = planes[nm][
                                        :, t * WPAD + coff : t * WPAD + coff + W
                                    ]
                                    nc.tensor.matmul(
                                        psums[t],
                                        (
                                            ident_sb[:, 2 * P : 3 * P].bitcast(mm_dt)
                                            if mm != "bf16"
                                            else ident_sb[:, 2 * P : 3 * P]
                                        )
                                        if skip_bands
                                        else band[:],
                                        rhs,
                                        start=first,
                                        stop=last,
                                    )
                        bias_ap = bias_sb[:, bias_col + o : bias_col + o + 1]
                        if c < 3:
                            out_nm = (
                                ["c1_0", "c1_1"][o]
                                if c == 0
                                else (f"c2_{o}" if c == 1 else f"c3_{o}")
                            )
                            for t in range(NT):
                                nc.scalar.add(
                                    pslice(out_nm, t, 2, 514), psums[t][:], bias_ap
                                )
                        else:
                            for t in range(NT):
                                st = c4_pool.tile([P, W], F32, tag="c4")
                                nc.scalar.add(st[:], psums[t][:], bias_ap)
                                rows = S if t < NT - 1 else H - S * (NT - 1)
                                nc.sync.dma_start(
                                    out=out_dram[o, S * t : S * t + rows, :],
                                    in_=st[HALO : HALO + rows, :],
                                )
                    # edge-zero the new planes (reference 'same' zero padding)
                    if c < 3:
                        outs = (
                            ["c1_0", "c1_1"]
                            if c == 0
                            else (
                                [f"c2_{i}" for i in range(4)]
                                if c == 1
                                else [f"c3_{i}" for i in range(8)]
                            )
                        )
                        for nm in outs:
                            nc.gpsimd.memset(msafe(planes[nm][0:HALO, 0:WPAD]), 0.0)
                            nc.gpsimd.memset(
                                msafe(
                                    planes[nm][
                                        p_hi_last:P, (NT - 1) * WPAD : NT * WPAD
                                    ]
                                ),
                                0.0,
                            )
                    bias_col += O

                # ---- remaining output channels from stored planes ----
                for ch in range(16, 48 if out_mode == 'full' else 16):
                    nm = CH_MAP[ch]
                    for t in range(NT):
                        rows = S if t < NT - 1 else H - S * (NT - 1)
                        src_ap = planes[nm][
                            HALO : HALO + rows, t * WPAD + 2 : t * WPAD + 514
                        ]
                        if mm == 'bf16':
                            nc.gpsimd.dma_start(
                                out=out_dram[ch, S * t : S * t + rows, :],
                                in_=src_ap,
                            )
                        else:
                            nc.sync.dma_start(
                                out=out_dram[ch, S * t : S * t + rows, :],
                                in_=src_ap.bitcast(F32),
                            )

    nc.compile()
    return nc


_NC_CACHE = None


def _get_nc():
    global _NC_CACHE
    if _NC_CACHE is None:
        _NC_CACHE = build_nc(mm=DEFAULT_MM)
    return _NC_CACHE


def _run(inputs, trace=False):
    inputs = {k: np.asarray(v) for k, v in inputs.items()}
    nc = _get_nc()
    wtab, ident, bias = _host_tables(inputs)
    feat = inputs["feature_in"].astype(np.float32)  # [8,1,512,512]
    n_cores = feat.shape[0]
    in_maps = [
        {"p": feat[b, 0], "ident": ident, "wtab": wtab, "bias": bias}
        for b in range(n_cores)
    ]
    res = bass_utils.run_bass_kernel_spmd(
        nc, in_maps, core_ids=list(range(n_cores)), trace=trace
    )
    out = np.stack([res.results[b]["out"] for b in range(n_cores)], axis=0)
    return out.astype(np.float32), res


def kernel(**inputs):
    return _run(inputs, trace=False)[0]

